# revision 16
# baseline (speedup 1.0000x reference)
"""Trainium2 Bass kernel for nn_End2EndTongueROI_Dynamic_NMS.

Key algebraic facts used (verified against the reference):
  - Greedy NMS always keeps the top-scored box first and fi=argmax(keep)=0,
    so the whole top-k/NMS tail reduces to argmax(score) over 8400 anchors.
  - score's /max(maskness) normalization and /32 mean are positive scalings
    shared by all anchors -> argmax-invariant -> dropped on device.
  - is_norm = (max(boxes_xywh) <= 1.2) is checked on HOST (inputs are
    uniform [0,1) so it always holds; if it ever fails we take the exact
    numpy fallback), so the device hardcodes the 640x center scaling.
  - Both resizes are linear: expressed as matmuls with exact f32 weight
    matrices replicated from jax.image.resize's compute_weight_mat.
  - The final rect is data-dependent but narrow, so each core computes its
    270-row shard restricted to a dynamic 128-row x 516-column window that
    covers the rect. Everything outside the window is exactly 0 in the
    reference output and the PJRT path pre-zeroes/donates output buffers,
    so only the window is written. A host-side check falls back to exact
    numpy if the rect ever exceeds the window.
  - The mask pipeline is computed only over the window's dependency cone:
    24 of 160 proto rows (H), a dynamic 28-wide column slice (W), a 2-matmul
    coef matvec, one matmul per resize leg, at exact jax f32 weights.
  - All box-derived window scalars (fb0..3, m, rw) are affine in the winner
    row followed by a single clamp, so ONE 37x8 matmul + 4 row ops computes
    them all at once.
  - The rect column mask is folded into the vww resize-matrix window (zeroed
    columns make the sigmoid-side product 0 < threshold), and the row mask
    is a per-partition {0,255} scale applied in one tensor_scalar.

Sharding: H0=2160 rows split 8 x 270. Score fusion + argmax tail is tiny and
fully replicated per core (no collectives needed).
"""
import numpy as np

import concourse.bacc as bacc
import concourse.bass as bass
import concourse.mybir as mybir
import concourse.tile as tile
from concourse import bass_isa, bass_utils

F32 = mybir.dt.float32
I32 = mybir.dt.int32
U32 = mybir.dt.uint32

N_CORES = 8
H0, W0 = 2160, 3840
IMGSZ = 640
MASK_THR = 0.72
NANCH, NC_COL = 8400, 37
ROWS = H0 // N_CORES          # 270 rows per core
SROWS = 82                    # s640 row window per core
MROWS = 24                    # m160 row window per core (padded)
WWIN = 516                    # output column window (6*86)
SWIN = 88                    # s-column window feeding WWIN
WW160 = 28                    # m160 column window feeding SWIN
RWIN = 128                    # output row window (one partition tile)
NPP = 66                      # anchors per partition (66*128 = 8448 >= 8400)
SX, SY = W0 / IMGSZ, H0 / IMGSZ          # 6.0, 3.375
# sentinel for the argmin-over-winners trick; power of two > NANCH so that
# idx - BIG is exact in f32
BIG = 16384.0


# ---------------------------------------------------------------------------
# host-side resize weights (exact replica of jax.image.resize bilinear)
# ---------------------------------------------------------------------------

def _weight_mat(in_size, out_size):
    dt = np.float32
    scale = dt(out_size / in_size)
    inv_scale = dt(1.0) / scale
    sample_f = (np.arange(out_size, dtype=dt) + dt(0.5)) * inv_scale - dt(0.5)
    x = np.abs(sample_f[None, :] - np.arange(in_size, dtype=dt)[:, None])
    w = np.maximum(dt(0), dt(1) - x).astype(dt)
    tot = w.sum(axis=0, keepdims=True).astype(dt)
    w = np.where(np.abs(tot) > 1000.0 * np.finfo(np.float32).eps,
                 w / np.where(tot != 0, tot, 1), 0).astype(dt)
    ok = (sample_f >= -0.5) & (sample_f <= in_size - 0.5)
    return np.where(ok[None, :], w, 0).astype(dt)


def _host_consts():
    """Constant tensors. Returns (shared, percore_list)."""
    Ah = _weight_mat(160, IMGSZ)      # [160, 640]
    Aw = _weight_mat(160, IMGSZ)      # [160, 640]
    Vh = _weight_mat(IMGSZ, H0)       # [640, 2160]
    Vw = _weight_mat(IMGSZ, W0)       # [640, 3840]

    # vwpad row i+1 = Vw row i (s-col i); zero guard rows at both ends so the
    # dynamic [SWIN, WWIN] slice at row m covers s-cols [m-1, m+86] with the
    # out-of-range ends contributing exactly zero.
    vwpad = np.zeros((642, W0), np.float32)
    vwpad[1:641] = Vw
    # AwT with the same one-column zero guard on both sides: awtp[w, j+1] =
    # Aw[w, j].  The dynamic [WW160, SWIN] slice at (ww, m) then aligns
    # column-for-column with the vwpad slice rows.
    awtp = np.zeros((160, 642), np.float32)
    awtp[:, 1:641] = Aw

    i128 = np.eye(128, dtype=np.float32)

    percore = []
    for c in range(N_CORES):
        r0 = ROWS * c
        vh_sl = Vh[:, r0:r0 + ROWS]
        nz = np.where(vh_sl.any(axis=1))[0]
        ra = min(int(nz.min()), IMGSZ - SROWS)
        r82 = np.ascontiguousarray(vh_sl[ra:ra + SROWS, :])     # [82, 270]

        ah_sl = Ah[:, ra:ra + SROWS]                            # [160, 82]
        nzh = np.where(ah_sl.any(axis=1))[0]
        ha = min(int(nzh.min()), 160 - MROWS)
        ahst = np.ascontiguousarray(ah_sl[ha:ha + MROWS, :])    # [24, 82]

        # W38: columns are affine forms of [winner pred row (37) | 1]:
        #  0: cx - w/2    1: cy - h/2    2: cx + w/2    3: cy + h/2
        #  4: cx - w/2 - 1.5 (m window)  5: SY*(cy - h/2) - r0 - 1.5 (rw)
        #  6: (cx - w/2 - 1.5)/4 - 1.375 (ww window)
        w38 = np.zeros((38, 8), np.float32)
        w38[0, 0] = 1.0; w38[2, 0] = -0.5
        w38[1, 1] = 1.0; w38[3, 1] = -0.5
        w38[0, 2] = 1.0; w38[2, 2] = 0.5
        w38[1, 3] = 1.0; w38[3, 3] = 0.5
        w38[0, 4] = 1.0; w38[2, 4] = -0.5; w38[37, 4] = -1.5
        w38[1, 5] = SY; w38[3, 5] = -0.5 * SY; w38[37, 5] = -(r0 + 1.5)
        w38[0, 6] = 0.25; w38[2, 6] = -0.125; w38[37, 6] = -1.75
        # crow: [hi | scale] rows of 8, packed as [1, 16]
        h8 = np.array([639, 639, 639, 639, 554, float(ROWS - RWIN), 132, 0],
                      np.float32)
        s8 = np.array([SX, SY, SX, SY, 1, 1, 1, 0], np.float32)
        crow = np.concatenate([h8, s8]).reshape(1, 16)

        r0c = np.full((1, 1), float(r0), np.float32)
        percore.append(dict(ra=ra, ha=ha, r82=r82, ahst=ahst, r0c=r0c,
                            w38=w38, crow=crow))

    shared = dict(awtp=awtp, vwpad=vwpad, i128=i128)
    return shared, percore


# ---------------------------------------------------------------------------
# device program (identical for all cores; per-core data comes via inputs)
# ---------------------------------------------------------------------------

def _build_nc(stage=99, reps=1, loop_n=0):
    nc = bacc.Bacc("TRN2", target_bir_lowering=False, debug=False,
                   enable_asserts=False, num_devices=N_CORES)

    d = {}
    d["predp"] = nc.dram_tensor("predp", [128, NPP * NC_COL], F32,
                                kind="ExternalInput")
    d["pred"] = nc.dram_tensor("pred", [NANCH, NC_COL], F32, kind="ExternalInput")
    d["xs"] = nc.dram_tensor("xs", [ROWS, W0 * 3], F32, kind="ExternalInput")
    d["protot"] = nc.dram_tensor("protot", [32, 160 * MROWS], F32,
                                 kind="ExternalInput")
    d["ahst"] = nc.dram_tensor("ahst", [MROWS, SROWS], F32, kind="ExternalInput")
    d["awtp"] = nc.dram_tensor("awtp", [160, 642], F32, kind="ExternalInput")
    d["r82"] = nc.dram_tensor("r82", [SROWS, ROWS], F32, kind="ExternalInput")
    d["vwpad"] = nc.dram_tensor("vwpad", [642, W0], F32, kind="ExternalInput")
    d["i128"] = nc.dram_tensor("i128", [128, 128], F32, kind="ExternalInput")
    d["w38"] = nc.dram_tensor("w38", [38, 8], F32, kind="ExternalInput")
    d["crow"] = nc.dram_tensor("crow", [1, 16], F32, kind="ExternalInput")
    d["r0c"] = nc.dram_tensor("r0c", [1, 1], F32, kind="ExternalInput")

    d["out"] = nc.dram_tensor("out", [ROWS, W0 * 3], F32, kind="ExternalOutput")
    d["meta"] = nc.dram_tensor("meta", [1, 8], F32, kind="ExternalOutput")

    with tile.TileContext(nc) as tc:
        if loop_n:
            with tc.For_i(0, loop_n, 1):
                _program(nc, tc, d, stage, 0)
        else:
            for rep in range(reps):
                _program(nc, tc, d, stage, rep)
    nc.compile()
    return nc


def _program(nc, tc, d, stage=99, rep=0):
    AF = mybir.ActivationFunctionType
    OP = mybir.AluOpType
    AX = mybir.AxisListType
    import contextlib
    ctx = contextlib.ExitStack()

    sb = ctx.enter_context(tc.tile_pool(name="sb", bufs=1))
    ps = ctx.enter_context(tc.tile_pool(name="ps", bufs=2,
                                        space=bass.MemorySpace.PSUM))

    _bias_cache = {}

    def cbias(val):
        if val not in _bias_cache:
            t = sb.tile([128, 1], F32, tag=f"cb{len(_bias_cache)}",
                        name=f"cb{rep}_{len(_bias_cache)}")
            nc.vector.memset(t[:, :], val)
            _bias_cache[val] = t
        return _bias_cache[val]

    def act(out_ap, in_ap, func, bias=0.0, scale=1.0):
        nparts = in_ap.shape[0]
        nc.scalar.activation(out_ap, in_ap, func,
                             bias=cbias(float(bias))[0:nparts, :],
                             scale=scale)

    def ts(out_ap, in_ap, s1, s2, op0, op1=None):
        nc.vector.tensor_scalar(out_ap, in_ap, s1, s2, op0,
                                *([] if op1 is None else [op1]))

    def tt(out_ap, a_ap, b_ap, op):
        nc.vector.tensor_tensor(out_ap, a_ap, b_ap, op)

    def tile1(tag, shape=(128, 1), dtype=F32):
        return sb.tile(list(shape), dtype, tag=tag, name=f"{tag}_{rep}")

    # ---------------- input DMAs (issue immediately) ----------------
    # pred packed [128, 2442]: halves split over the two HWDGE queues
    P2 = tile1("P2", (128, NPP * NC_COL))
    nc.sync.dma_start(P2[0:64, :], d["predp"].ap()[0:64, :])
    nc.scalar.dma_start(P2[64:128, :], d["predp"].ap()[64:128, :])
    w38 = tile1("w38", (38, 8))
    nc.scalar.dma_start(w38[:, :], d["w38"].ap())
    crow = tile1("crow", (1, 16))
    nc.scalar.dma_start(crow[:, :], d["crow"].ap())
    i128 = tile1("i128", (128, 128))
    nc.scalar.dma_start(i128[:, :], d["i128"].ap())
    ahst = tile1("ahst", (MROWS, SROWS))
    nc.scalar.dma_start(ahst[:, :], d["ahst"].ap())
    r0c = tile1("r0c", (1, 1))
    nc.scalar.dma_start(r0c[:, :], d["r0c"].ap())

    # engine-generated constants (no DMA)
    ones1 = tile1("ones1", (1, 128))
    nc.vector.memset(ones1[:, :], 1.0)
    id1 = tile1("id1", (1, 1))
    nc.vector.memset(id1[:, :], 1.0)
    row1x = tile1("row1x", (1, 38))
    nc.vector.memset(row1x[0:1, 37:38], 1.0)
    pio1i = tile1("pio1i", (128, 1), I32)
    nc.gpsimd.iota(pio1i[:, :], pattern=[[0, 1]], base=0, channel_multiplier=1)
    pio1f = tile1("pio1f")
    nc.vector.tensor_copy(pio1f[:, :], pio1i[:, :])
    pioei = tile1("pioei", (128, 1), I32)
    nc.gpsimd.iota(pioei[:, :], pattern=[[0, 1]], base=-int(BIG),
                   channel_multiplier=NPP)
    pioef = tile1("pioef")
    nc.vector.tensor_copy(pioef[:, :], pioei[:, :])
    xii = tile1("xii", (SWIN, WWIN), I32)
    nc.gpsimd.iota(xii[:, :], pattern=[[1, WWIN]], base=0, channel_multiplier=0)
    xif = tile1("xif", (SWIN, WWIN))
    nc.vector.tensor_copy(xif[:, :], xii[:, :])

    # ---------------- stage S: score fusion + argmax ----------------
    P3 = P2[:, :].rearrange("p (n c) -> p n c", c=NC_COL)   # [128, 66, 37]

    sg = tile1("sg", (128, NPP))
    act(sg[:, :], P3[:, :, 4], AF.Sigmoid)
    s2 = tile1("s2", (128, NPP))
    ts(s2[:, :], sg[:, :], -0.5, 0.0, OP.add, OP.max)       # relu(sig-0.5)
    ts(s2[:, :], s2[:, :], 0.001, None, OP.add)

    mk = tile1("mk", (128, NPP))
    nc.vector.tensor_reduce(mk[:, :], P3[:, :, 5:NC_COL], AX.X, OP.add,
                            apply_absolute_value=True)

    dxa = tile1("dxa", (128, NPP))
    dya = tile1("dya", (128, NPP))
    act(dxa[:, :], P3[:, :, 0], AF.Abs, bias=-320.0, scale=640.0)
    act(dya[:, :], P3[:, :, 1], AF.Abs, bias=-320.0, scale=640.0)
    uxy = tile1("uxy", (128, NPP))
    tt(uxy[:, :], dxa[:, :], dya[:, :], OP.add)
    # 0.5 + 0.5*clamp(1 - uxy/640, 0, 1) == max(1 - uxy/1280, 0.5)
    cwf = tile1("cwf", (128, NPP))
    ts(cwf[:, :], uxy[:, :], -0.5 / 640.0, 1.0, OP.mult, OP.add)
    ts(cwf[:, :], cwf[:, :], 0.5, None, OP.max)

    score = tile1("score", (128, NPP))
    tt(score[:, :], s2[:, :], mk[:, :], OP.mult)
    tt(score[:, :], score[:, :], cwf[:, :], OP.mult)

    vmax8 = tile1("vmax8", (128, 8))
    vidx8 = tile1("vidx8", (128, 8), U32)
    nc.vector.max_with_indices(vmax8[:, :], vidx8[:, :], score[:, :])

    # winner anchor = min over {p : vmax[p] == global max} of (66p + idx[p]),
    # done in the transposed [1,128] domain with the -BIG encoding.
    afe = tile1("afe")
    nc.vector.tensor_copy(afe[:, :], vidx8[:, 0:1])
    ts(afe[:, :], afe[:, :], pioef[:, :], None, OP.add)     # 66p + idx - BIG
    pmA = ps.tile([1, 128], F32, tag="ps", name=f"pmA{rep}")
    nc.tensor.transpose(pmA[:, :], vmax8[:, 0:1], i128[:, :])
    pmB = ps.tile([1, 128], F32, tag="ps", name=f"pmB{rep}")
    nc.tensor.transpose(pmB[:, :], afe[:, :], i128[:, :])
    m11 = tile1("m11", (1, 1))
    nc.vector.tensor_reduce(m11[0:1, :], pmA[:, :], AX.X, OP.max)
    wm = tile1("wm", (1, 128))
    ts(wm[0:1, :], pmA[:, :], m11[0:1, :], None, OP.is_ge)
    cand = tile1("cand", (1, 128))
    tt(cand[0:1, :], pmB[:, :], wm[0:1, :], OP.mult)
    a_f = tile1("a_f", (1, 1))
    nc.vector.tensor_reduce(a_f[0:1, :], cand[0:1, :], AX.X, OP.min)
    ts(a_f[0:1, :], a_f[0:1, :], BIG, None, OP.add)
    a_i = tile1("a_i", (1, 1), I32)
    nc.vector.tensor_copy(a_i[0:1, :], a_f[0:1, :])

    if stage <= 1:
        metas = tile1("metas", (1, 8))
        nc.vector.memset(metas[:, :], 0.0)
        nc.vector.tensor_copy(metas[0:1, 0:1], a_f[0:1, :])
        nc.sync.dma_start(d["meta"].ap(), metas[:, :])
        ctx.close()
        return

    # ---------------- stage G: gather winner row; box -> windows ----------
    with nc.sync.register(f"aoff{rep}") as areg:
        nc.sync.reg_load(areg, a_i[0:1, 0:1])
        aoff = nc.sync.snap(areg, min_val=0, max_val=NANCH - 1)
        nc.sync.dma_start(row1x[:, 0:NC_COL],
                          d["pred"].ap()[bass.ds(aoff, 1), :])

    t38 = tile1("t38", (38, 1))
    psT = ps.tile([38, 1], F32, tag="ps", name=f"psT{rep}")
    nc.tensor.transpose(psT[:, :], row1x[:, :], id1[:, :])
    nc.scalar.copy(t38[:, :], psT[:, :])
    psC = ps.tile([32, 1], F32, tag="ps", name=f"psC{rep}")
    nc.tensor.transpose(psC[:, :], row1x[:, 5:NC_COL], id1[:, :])
    coefT = tile1("coefT", (32, 1))
    nc.scalar.copy(coefT[:, :], psC[:, :])
    coefT = coefT[:, :]

    psV = ps.tile([1, 8], F32, tag="ps", name=f"psV{rep}")
    nc.tensor.matmul(psV[:, :], t38[:, :], w38[:, :], start=True, stop=True)
    # vr = clamp(psV, 0, h8) * s8   (bias folded into w38 row 37)
    vr = tile1("vr", (1, 8))
    ts(vr[0:1, :], psV[:, :], 0.0, None, OP.max)
    tt(vr[0:1, :], vr[0:1, :], crow[0:1, 0:8], OP.min)
    tt(vr[0:1, :], vr[0:1, :], crow[0:1, 8:16], OP.mult)
    # vr = [fb0, fb1, fb2, fb3, m_pre, rw_pre, ww_pre, 0]
    # (ww from unrounded m_pre is safe: it can only yield ww or ww-1, both
    #  inside the 4-col slack.)

    ri2 = tile1("ri2", (1, 2), I32)                          # [m_i, rw_i]
    nc.vector.tensor_copy(ri2[0:1, :], vr[0:1, 4:6])
    rf2 = tile1("rf2", (1, 2))
    nc.vector.tensor_copy(rf2[0:1, :], ri2[0:1, :])
    c0_i = tile1("c0_i", (1, 1), I32)
    ts(c0_i[0:1, :], ri2[0:1, 0:1], 6, None, OP.mult)
    ww_i = tile1("ww_i", (1, 1), I32)
    nc.vector.tensor_copy(ww_i[0:1, :], vr[0:1, 6:7])

    # vrow2 = [fb0, fb1, fb2, fb3, c0, rw+r0, a, 0] -> meta + broadcast
    vrow2 = tile1("vrow2", (1, 8))
    nc.vector.memset(vrow2[:, :], 0.0)
    nc.vector.tensor_copy(vrow2[0:1, 0:4], vr[0:1, 0:4])
    nc.vector.tensor_copy(vrow2[0:1, 4:5], c0_i[0:1, :])
    tt(vrow2[0:1, 5:6], rf2[0:1, 1:2], r0c[0:1, :], OP.add)
    nc.vector.tensor_copy(vrow2[0:1, 6:7], a_f[0:1, :])
    nc.sync.dma_start(d["meta"].ap(), vrow2[:, :])

    psF = ps.tile([128, 8], F32, tag="ps", name=f"psF{rep}")
    nc.tensor.matmul(psF[:, :], ones1[:, :], vrow2[:, :], start=True, stop=True)
    fbB = tile1("fbB", (128, 8))
    nc.scalar.copy(fbB[:, :], psF[:, :])

    riog = tile1("riog")                       # global row index per partition
    ts(riog[:, :], pio1f[:, :], fbB[:, 5:6], None, OP.add)
    rma = tile1("rma")
    rmb = tile1("rmb")
    ts(rma[:, :], riog[:, :], fbB[:, 1:2], None, OP.is_ge)
    ts(rmb[:, :], riog[:, :], fbB[:, 3:4], 255.0, OP.is_lt, OP.mult)
    rm255 = tile1("rm255")
    tt(rm255[:, :], rma[:, :], rmb[:, :], OP.mult)

    if stage <= 2:
        ctx.close()
        return

    # ---------------- stage M: windowed mask pipeline ----------------
    protosw = tile1("protosw", (32, WW160 * MROWS))
    awW = tile1("awW", (WW160, SWIN))
    vww = tile1("vww", (SWIN, WWIN))
    r82w = tile1("r82w", (SROWS, RWIN))
    xw = tile1("xw", (128, WWIN * 3))
    greg_ctx = contextlib.ExitStack()
    mreg = greg_ctx.enter_context(nc.gpsimd.register(f"mo{rep}"))
    wreg = greg_ctx.enter_context(nc.gpsimd.register(f"wo{rep}"))
    creg = greg_ctx.enter_context(nc.gpsimd.register(f"co_{rep}"))
    rreg = greg_ctx.enter_context(nc.gpsimd.register(f"ro{rep}"))
    nc.gpsimd.reg_load(mreg, ri2[0:1, 0:1])
    nc.gpsimd.reg_load(wreg, ww_i[0:1, 0:1])
    nc.gpsimd.reg_load(creg, c0_i[0:1, 0:1])
    nc.gpsimd.reg_load(rreg, ri2[0:1, 1:2])
    mo = nc.gpsimd.snap(mreg, min_val=0, max_val=554)
    wo = nc.gpsimd.snap(wreg, min_val=0, max_val=160 - WW160)
    co = nc.gpsimd.snap(creg, min_val=0, max_val=W0 - WWIN)
    ro = nc.gpsimd.snap(rreg, min_val=0, max_val=ROWS - RWIN)
    # proto is staged w-major ([32, w160, h24]) so this is 32 descriptors
    nc.gpsimd.dma_start(
        protosw[:, :].rearrange("c (w h) -> c w h", h=MROWS),
        d["protot"].ap().rearrange("c (w h) -> c w h", h=MROWS)
        [:, bass.ds(wo, WW160), :])
    nc.gpsimd.dma_start(awW[:, :],
                        d["awtp"].ap()[bass.ds(wo, WW160), bass.ds(mo, SWIN)])
    nc.gpsimd.dma_start(vww[:, :],
                        d["vwpad"].ap()[bass.ds(mo, SWIN), bass.ds(co, WWIN)])
    nc.gpsimd.dma_start(r82w[:, :], d["r82"].ap()[:, bass.ds(ro, RWIN)])
    nc.gpsimd.dma_start(
        xw[:, :].rearrange("p (w c) -> p w c", c=3),
        d["xs"].ap().rearrange("r (w c) -> r w c", c=3)
        [bass.ds(ro, RWIN), bass.ds(co, WWIN), :])

    # coef matvec over the window: m160wT flat [1, (w28 h24)]
    psM = ps.tile([1, WW160 * MROWS], F32, tag="psM", name=f"psM{rep}", bufs=1)
    nc.tensor.matmul(psM[0:1, 0:512], coefT, protosw[:, 0:512],
                     start=True, stop=True)
    nc.tensor.matmul(psM[0:1, 512:WW160 * MROWS], coefT,
                     protosw[:, 512:WW160 * MROWS], start=True, stop=True)
    m160wf = tile1("m160wf", (1, WW160 * MROWS))
    nc.scalar.copy(m160wf[:, :], psM[:, :])
    m160wT = tile1("m160wT", (WW160, MROWS))
    nc.sync.dma_start(
        m160wT[:, :],
        m160wf[:, :].rearrange("q (w h) -> (q w) h", h=MROWS))

    # step A: contract w:  sA[h24, i88] = sum_w m160wT[w, h] * awW[w, i]
    psA = ps.tile([MROWS, SWIN], F32, tag="ps", name=f"psA{rep}")
    nc.tensor.matmul(psA[:, :], m160wT[:, :], awW[:, :], start=True, stop=True)
    sA = tile1("sA", (MROWS, SWIN))
    nc.scalar.copy(sA[:, :], psA[:, :])

    # step B: contract h:  m640T[i88, j82] = sum_h sA[h, i] * ahst[h, j]
    psB = ps.tile([SWIN, SROWS], F32, tag="ps", name=f"psB{rep}")
    nc.tensor.matmul(psB[:, :], sA[:, :], ahst[:, :], start=True, stop=True)
    s_winT = tile1("s_winT", (SWIN, SROWS))
    act(s_winT[:, :], psB[:, :], AF.Sigmoid)

    if stage <= 3:
        greg_ctx.close()
        ctx.close()
        return

    # ---------------- stage O: threshold + rect + multiply ----------------
    # column rect mask folded into the vww window (zeroed columns give
    # sigmoid-product 0 < MASK_THR)
    xcol = tile1("xcol", (SWIN, WWIN))
    ts(xcol[:, :], xif[:, :], fbB[0:SWIN, 4:5], None, OP.add)
    cma = tile1("cma", (SWIN, WWIN))
    ts(cma[:, :], xcol[:, :], fbB[0:SWIN, 0:1], None, OP.is_ge)
    cmb = tile1("cmb", (SWIN, WWIN))
    ts(cmb[:, :], xcol[:, :], fbB[0:SWIN, 2:3], None, OP.is_lt)
    colm = tile1("colm", (SWIN, WWIN))
    tt(colm[:, :], cma[:, :], cmb[:, :], OP.mult)
    vwwm = tile1("vwwm", (SWIN, WWIN))
    tt(vwwm[:, :], vww[:, :], colm[:, :], OP.mult)

    # step X: contract i:  tX[j82, col] = sum_i s_winT[i, j] * vwwm[i, col]
    psX = ps.tile([SROWS, WWIN], F32, tag="psM", name=f"psX{rep}", bufs=1)
    nc.tensor.matmul(psX[:, 0:512], s_winT[:, :], vwwm[:, 0:512],
                     start=True, stop=True)
    nc.tensor.matmul(psX[:, 512:WWIN], s_winT[:, :], vwwm[:, 512:WWIN],
                     start=True, stop=True)
    sX = tile1("sX", (SROWS, WWIN))
    nc.scalar.copy(sX[:, :], psX[:, :])

    # step W: contract j:  m_orig[r128, col] = sum_j r82w[j, r] * tX[j, col]
    psW = ps.tile([RWIN, WWIN], F32, tag="psW", name=f"psW{rep}", bufs=1)
    nc.tensor.matmul(psW[:, 0:512], r82w[:, :], sX[:, 0:512],
                     start=True, stop=True)
    nc.tensor.matmul(psW[:, 512:WWIN], r82w[:, :], sX[:, 512:WWIN],
                     start=True, stop=True)
    bm3 = tile1("bm3", (RWIN, WWIN))
    ts(bm3[:, :], psW[:, :], MASK_THR, rm255[:, :], OP.is_gt, OP.mult)

    res = tile1("res", (RWIN, 3 * WWIN))
    res3 = res[:, :].rearrange("p (w c) -> p w c", c=3)
    xw3 = xw[:, :].rearrange("p (w c) -> p w c", c=3)
    for ch in range(3):
        tt(res3[:, :, ch], xw3[:, :, ch], bm3[:, :], OP.mult)

    nc.gpsimd.dma_start(
        d["out"].ap().rearrange("r (w c) -> r w c", c=3)
        [bass.ds(ro, RWIN), bass.ds(co, WWIN), :],
        res[:, :].rearrange("p (w c) -> p w c", c=3))

    greg_ctx.close()
    ctx.close()


# ---------------------------------------------------------------------------
# host orchestration
# ---------------------------------------------------------------------------

_NC_CACHE = None


def _get_nc():
    global _NC_CACHE
    if _NC_CACHE is None:
        _NC_CACHE = _build_nc()
    return _NC_CACHE


def _make_in_maps(x_raw, pred2, proto2, shared, percore):
    predp = np.zeros((128, NPP * NC_COL), np.float32)
    predp.reshape(-1)[:NANCH * NC_COL] = pred2.reshape(-1)
    in_maps = []
    for c in range(N_CORES):
        pc = percore[c]
        ha = pc["ha"]
        xs_cl = np.ascontiguousarray(
            x_raw[0, :, ROWS * c:ROWS * (c + 1), :].transpose(1, 2, 0)
        ).reshape(ROWS, W0 * 3)
        protot = np.ascontiguousarray(
            proto2[:, ha:ha + MROWS, :].transpose(0, 2, 1)
        ).reshape(32, 160 * MROWS)
        in_maps.append({
            "predp": predp,
            "pred": pred2,
            "xs": xs_cl,
            "protot": protot,
            "ahst": pc["ahst"],
            "awtp": shared["awtp"],
            "r82": pc["r82"],
            "vwpad": shared["vwpad"],
            "i128": shared["i128"],
            "w38": pc["w38"],
            "crow": pc["crow"],
            "r0c": pc["r0c"],
        })
    return in_maps


def _numpy_fallback(x_raw, pred, proto):
    """Exact slow-path reference (only used if the rect exceeds the device
    windows, which cannot happen for in-distribution inputs)."""
    p = pred[0]
    boxes, cls, coef = p[:, :4], p[:, 4], p[:, 5:]
    s1 = np.maximum(1.0 / (1.0 + np.exp(-cls)) - 0.5, 0) + np.float32(0.001)
    mk = np.abs(coef).sum(-1)
    f = np.float32(640.0 if boxes.max() <= 1.2 else 1.0)
    dxdy = np.abs(boxes[:, :2] * f - 320.0) / 320.0
    cw = np.maximum(1.0 - 0.5 * (dxdy[:, 0] + dxdy[:, 1]), 0.0)
    a = int(np.argmax(s1 * mk * (0.5 + 0.5 * cw)))
    fcoef = coef[a]
    cx, cy, w, h = boxes[a]
    xyxy = np.clip(np.array([cx - w / 2, cy - h / 2, cx + w / 2, cy + h / 2],
                            np.float32), 0.0, IMGSZ - 1)
    fb = xyxy * np.array([W0 / IMGSZ, H0 / IMGSZ, W0 / IMGSZ, H0 / IMGSZ],
                         np.float32)
    Ah = _weight_mat(160, IMGSZ)
    Aw = _weight_mat(160, IMGSZ)
    Vh = _weight_mat(IMGSZ, H0)
    Vw = _weight_mat(IMGSZ, W0)
    m160 = (fcoef @ proto[0].reshape(32, -1)).reshape(160, 160)
    m640 = Ah.T @ m160 @ Aw
    s640 = 1.0 / (1.0 + np.exp(-m640))
    m_orig = (Vh.T @ s640 @ Vw).astype(np.float32)
    ys = np.arange(H0, dtype=np.float32)[:, None]
    xs = np.arange(W0, dtype=np.float32)[None, :]
    rect = (xs >= fb[0]) & (xs < fb[2]) & (ys >= fb[1]) & (ys < fb[3])
    bm = ((m_orig > MASK_THR) & rect).astype(np.float32)
    return (np.clip(x_raw * 255.0, 0.0, 255.0) * bm[None, None]).astype(np.float32)


def _covered(metas):
    """Check every rect pixel lies inside each core's written window.
    meta = [fb0, fb1, fb2, fb3, c0, rw + r0, a, 0]"""
    fb0, fb1, fb2, fb3 = metas[0][0], metas[0][1], metas[0][2], metas[0][3]
    if fb2 <= fb0 or fb3 <= fb1:
        return True
    c0 = metas[0][4]
    cols = np.arange(W0, dtype=np.float32)
    csel = (cols >= fb0) & (cols < fb2)
    if csel.any():
        lo, hi = np.where(csel)[0][[0, -1]]
        if not (c0 <= lo and hi < c0 + WWIN):
            return False
    rows = np.arange(H0, dtype=np.float32)
    rsel = (rows >= fb1) & (rows < fb3)
    for c in range(N_CORES):
        sel = rsel[ROWS * c:ROWS * (c + 1)]
        if sel.any():
            rw = metas[c][5] - ROWS * c
            lo, hi = np.where(sel)[0][[0, -1]]
            if not (rw <= lo and hi < rw + RWIN):
                return False
    return True


def kernel(x_raw, pred, proto):
    x_raw = np.ascontiguousarray(np.asarray(x_raw, dtype=np.float32))
    pred = np.ascontiguousarray(np.asarray(pred, dtype=np.float32))
    proto = np.ascontiguousarray(np.asarray(proto, dtype=np.float32))

    if float(pred[0, :, :4].max()) > 1.2:
        # device hardcodes the is_norm=True 640x scaling
        return _numpy_fallback(x_raw, pred, proto)

    nc = _get_nc()
    shared, percore = _host_consts()
    pred2 = np.ascontiguousarray(pred[0])
    proto2 = proto[0]
    in_maps = _make_in_maps(x_raw, pred2, proto2, shared, percore)

    res = bass_utils.run_bass_kernel_spmd(nc, in_maps,
                                          core_ids=list(range(N_CORES)))

    metas = [res.results[c]["meta"][0] for c in range(N_CORES)]
    if not _covered(metas):
        return _numpy_fallback(x_raw, pred, proto)

    out = np.concatenate(
        [res.results[c]["out"].reshape(ROWS, W0, 3) for c in range(N_CORES)],
        axis=0)                                   # [2160, 3840, 3]
    return np.ascontiguousarray(out.transpose(2, 0, 1))[None]


if __name__ == "__main__":
    import jax
    with jax.default_device(jax.devices("cpu")[0]):
        import reference as R
        inputs = R.setup_inputs()
        inputs = {k: np.asarray(v) for k, v in inputs.items()}
    out = kernel(**inputs)
    ref = np.load("/tmp/ref_out.npy")
    print("absmax:", np.abs(out - ref).max())


# revision 42
# speedup vs baseline: 1.5992x; 1.5992x over previous
"""Trainium2 Bass kernel for nn_End2EndTongueROI_Dynamic_NMS.

Key algebraic facts used (verified against the reference):
  - Greedy NMS always keeps the top-scored box first and fi=argmax(keep)=0,
    so the whole top-k/NMS tail reduces to argmax(score) over 8400 anchors.
  - score's /max(maskness) normalization and /32 mean are positive scalings
    shared by all anchors -> argmax-invariant -> dropped on device.
  - is_norm = (max(boxes_xywh) <= 1.2) is checked on HOST (inputs are
    uniform [0,1) so it always holds; if it ever fails we take the exact
    numpy fallback), so the device hardcodes the 640x center scaling.
  - Both resizes are linear: expressed as matmuls with exact f32 weight
    matrices replicated from jax.image.resize's compute_weight_mat.
  - The final rect is data-dependent but narrow, so each core computes its
    270-row shard restricted to a dynamic 128-row x 516-column window that
    covers the rect. Everything outside the window is exactly 0 in the
    reference output and the PJRT path pre-zeroes/donates output buffers,
    so only the window is written. A host-side check falls back to exact
    numpy if the rect ever exceeds the window.
  - The mask pipeline is computed only over the window's dependency cone:
    24 of 160 proto rows (H), a dynamic 28-wide column slice (W), a 2-matmul
    coef matvec, one matmul per resize leg, at exact jax f32 weights.
  - All box-derived window scalars (fb0..3, m, rw) are affine in the winner
    row followed by a single clamp, so ONE 37x8 matmul + 4 row ops computes
    them all at once.
  - The rect column mask is folded into the vww resize-matrix window (zeroed
    columns make the sigmoid-side product 0 < threshold), and the row mask
    is a per-partition {0,255} scale applied in one tensor_scalar.

Sharding: H0=2160 rows split 8 x 270. Score fusion + argmax tail is tiny and
fully replicated per core (no collectives needed).
"""
import numpy as np

import concourse.bacc as bacc
import concourse.bass as bass
import concourse.mybir as mybir
import concourse.tile as tile
from concourse import bass_isa, bass_utils

F32 = mybir.dt.float32
BF16 = mybir.dt.bfloat16
I32 = mybir.dt.int32
U32 = mybir.dt.uint32

N_CORES = 8
H0, W0 = 2160, 3840
IMGSZ = 640
MASK_THR = 0.72
NANCH, NC_COL = 8400, 37
ROWS = H0 // N_CORES          # 270 rows per core
SROWS = 82                    # s640 row window per core
MROWS = 24                    # m160 row window per core (padded)
# the rect is tiny for in-distribution inputs (box w,h ~ U[0,1) so the
# scaled rect is < 6x4 px); the host _covered check falls back to exact
# numpy if it ever exceeds these windows.
WWIN = 18                     # output column window (6*3)
SWIN = 5                      # s-column window feeding WWIN
WW160 = 8                     # m160 column window feeding SWIN
RWIN = 8                      # output row window
NPP = 66                      # anchors per partition (66*128 = 8448 >= 8400)
SX, SY = W0 / IMGSZ, H0 / IMGSZ          # 6.0, 3.375
# sentinel for the argmin-over-winners trick; power of two > NANCH so that
# idx - BIG is exact in f32
BIG = 16384.0


# ---------------------------------------------------------------------------
# host-side resize weights (exact replica of jax.image.resize bilinear)
# ---------------------------------------------------------------------------

def _weight_mat(in_size, out_size):
    dt = np.float32
    scale = dt(out_size / in_size)
    inv_scale = dt(1.0) / scale
    sample_f = (np.arange(out_size, dtype=dt) + dt(0.5)) * inv_scale - dt(0.5)
    x = np.abs(sample_f[None, :] - np.arange(in_size, dtype=dt)[:, None])
    w = np.maximum(dt(0), dt(1) - x).astype(dt)
    tot = w.sum(axis=0, keepdims=True).astype(dt)
    w = np.where(np.abs(tot) > 1000.0 * np.finfo(np.float32).eps,
                 w / np.where(tot != 0, tot, 1), 0).astype(dt)
    ok = (sample_f >= -0.5) & (sample_f <= in_size - 0.5)
    return np.where(ok[None, :], w, 0).astype(dt)


def _host_consts():
    """Constant tensors. Returns (shared, percore_list)."""
    Ah = _weight_mat(160, IMGSZ)      # [160, 640]
    Aw = _weight_mat(160, IMGSZ)      # [160, 640]
    Vh = _weight_mat(IMGSZ, H0)       # [640, 2160]
    Vw = _weight_mat(IMGSZ, W0)       # [640, 3840]

    # vwpad row i+1 = Vw row i (s-col i); zero guard rows at both ends so the
    # dynamic [SWIN, WWIN] slice at row m covers s-cols [m-1, m+86] with the
    # out-of-range ends contributing exactly zero.
    vwpad = np.zeros((642, W0), np.float32)
    vwpad[1:641] = Vw
    # AwT with the same one-column zero guard on both sides: awtp[w, j+1] =
    # Aw[w, j].  The dynamic [WW160, SWIN] slice at (ww, m) then aligns
    # column-for-column with the vwpad slice rows.
    awtp = np.zeros((160, 642), np.float32)
    awtp[:, 1:641] = Aw

    i128 = np.eye(128, dtype=np.float32)
    xiota = np.ascontiguousarray(
        np.broadcast_to(np.arange(WWIN, dtype=np.float32), (SWIN, WWIN)))

    percore = []
    for c in range(N_CORES):
        r0 = ROWS * c
        vh_sl = Vh[:, r0:r0 + ROWS]
        nz = np.where(vh_sl.any(axis=1))[0]
        ra = min(int(nz.min()), IMGSZ - SROWS)
        r82 = np.ascontiguousarray(vh_sl[ra:ra + SROWS, :])     # [82, 270]

        ah_sl = Ah[:, ra:ra + SROWS]                            # [160, 82]
        nzh = np.where(ah_sl.any(axis=1))[0]
        ha = min(int(nzh.min()), 160 - MROWS)
        ahst = np.ascontiguousarray(ah_sl[ha:ha + MROWS, :])    # [24, 82]

        # W38: columns are affine forms of [winner pred row (37) | 1]:
        #  0: cx - w/2    1: cy - h/2    2: cx + w/2    3: cy + h/2
        #  4: cx - w/2 - 1.5 (m window)  5: SY*(cy - h/2) - r0 - 1.5 (rw)
        #  6: (cx - w/2 - 1.5)/4 - 1.375 (ww window)
        w38 = np.zeros((38, 8), np.float32)
        w38[0, 0] = 1.0; w38[2, 0] = -0.5
        w38[1, 1] = 1.0; w38[3, 1] = -0.5
        w38[0, 2] = 1.0; w38[2, 2] = 0.5
        w38[1, 3] = 1.0; w38[3, 3] = 0.5
        w38[0, 4] = 1.0; w38[2, 4] = -0.5; w38[37, 4] = -1.5
        w38[1, 5] = SY; w38[3, 5] = -0.5 * SY; w38[37, 5] = -(r0 + 1.5)
        w38[0, 6] = 0.25; w38[2, 6] = -0.125; w38[37, 6] = -1.75
        # crow: [hi | scale] rows of 8, packed as [1, 16]
        h8 = np.array([639, 639, 639, 639, (W0 - WWIN) / 6.0,
                       float(ROWS - RWIN), float(160 - WW160), 0],
                      np.float32)
        s8 = np.array([SX, SY, SX, SY, 1, 1, 1, 0], np.float32)
        crow = np.concatenate([h8, s8]).reshape(1, 16)

        r0c = np.full((1, 1), float(r0), np.float32)
        percore.append(dict(ra=ra, ha=ha, r82=r82, ahst=ahst, r0c=r0c,
                            w38=w38, crow=crow))

    shared = dict(awtp=awtp, vwpad=vwpad, i128=i128, xiota=xiota)
    return shared, percore


# ---------------------------------------------------------------------------
# device program (identical for all cores; per-core data comes via inputs)
# ---------------------------------------------------------------------------

def _build_nc(stage=99, reps=1, loop_n=0):
    nc = bacc.Bacc("TRN2", target_bir_lowering=False, debug=False,
                   enable_asserts=False, num_devices=N_CORES)

    d = {}
    d["predp"] = nc.dram_tensor("predp", [128, NPP * NC_COL], BF16,
                                kind="ExternalInput")
    d["pred"] = nc.dram_tensor("pred", [NANCH, NC_COL], F32, kind="ExternalInput")
    d["xs"] = nc.dram_tensor("xs", [ROWS, W0 * 3], F32, kind="ExternalInput")
    d["protot"] = nc.dram_tensor("protot", [32, 160 * MROWS], F32,
                                 kind="ExternalInput")
    d["ahst"] = nc.dram_tensor("ahst", [MROWS, SROWS], F32, kind="ExternalInput")
    d["awtp"] = nc.dram_tensor("awtp", [160, 642], F32, kind="ExternalInput")
    d["r82"] = nc.dram_tensor("r82", [SROWS, ROWS], F32, kind="ExternalInput")
    d["vwpad"] = nc.dram_tensor("vwpad", [642, W0], F32, kind="ExternalInput")
    d["i128"] = nc.dram_tensor("i128", [128, 128], F32, kind="ExternalInput")
    d["xiota"] = nc.dram_tensor("xiota", [SWIN, WWIN], F32,
                                kind="ExternalInput")
    d["w38"] = nc.dram_tensor("w38", [38, 8], F32, kind="ExternalInput")
    d["crow"] = nc.dram_tensor("crow", [1, 16], F32, kind="ExternalInput")
    d["r0c"] = nc.dram_tensor("r0c", [1, 1], F32, kind="ExternalInput")

    d["out"] = nc.dram_tensor("out", [ROWS, W0 * 3], F32, kind="ExternalOutput")
    d["meta"] = nc.dram_tensor("meta", [1, 8], F32, kind="ExternalOutput")

    with tile.TileContext(nc) as tc:
        if loop_n:
            with tc.For_i(0, loop_n, 1):
                _program(nc, tc, d, stage, 0)
        else:
            for rep in range(reps):
                _program(nc, tc, d, stage, rep)
    nc.compile()
    return nc


def _program(nc, tc, d, stage=99, rep=0):
    AF = mybir.ActivationFunctionType
    OP = mybir.AluOpType
    AX = mybir.AxisListType
    import contextlib
    ctx = contextlib.ExitStack()

    sb = ctx.enter_context(tc.tile_pool(name="sb", bufs=1))
    ps = ctx.enter_context(tc.tile_pool(name="ps", bufs=2,
                                        space=bass.MemorySpace.PSUM))

    _bias_cache = {}

    def cbias(val):
        if val not in _bias_cache:
            t = sb.tile([128, 1], F32, tag=f"cb{len(_bias_cache)}",
                        name=f"cb{rep}_{len(_bias_cache)}")
            nc.vector.memset(t[:, :], val)
            _bias_cache[val] = t
        return _bias_cache[val]

    def act(out_ap, in_ap, func, bias=0.0, scale=1.0):
        nparts = in_ap.shape[0]
        nc.scalar.activation(out_ap, in_ap, func,
                             bias=cbias(float(bias))[0:nparts, :],
                             scale=scale)

    def ts(out_ap, in_ap, s1, s2, op0, op1=None):
        nc.vector.tensor_scalar(out_ap, in_ap, s1, s2, op0,
                                *([] if op1 is None else [op1]))

    def tt(out_ap, a_ap, b_ap, op):
        nc.vector.tensor_tensor(out_ap, a_ap, b_ap, op)

    def tile1(tag, shape=(128, 1), dtype=F32):
        return sb.tile(list(shape), dtype, tag=tag, name=f"{tag}_{rep}")

    # ---------------- input DMAs (issue immediately) ----------------
    # pred packed [128, 2442] bf16: halves split over the two HWDGE queues.
    # bf16 is safe: only the argmax winner matters and the winner's exact
    # f32 row is re-gathered; the host falls back if the bf16-quantized
    # argmax could differ.
    P2 = tile1("P2", (128, NPP * NC_COL), BF16)
    nc.sync.dma_start(P2[0:64, :], d["predp"].ap()[0:64, :])
    nc.scalar.dma_start(P2[64:128, :], d["predp"].ap()[64:128, :])
    w38 = tile1("w38", (38, 8))
    nc.scalar.dma_start(w38[:, :], d["w38"].ap())
    crow = tile1("crow", (1, 16))
    nc.scalar.dma_start(crow[:, :], d["crow"].ap())
    i128 = tile1("i128", (128, 128))
    nc.scalar.dma_start(i128[:, :], d["i128"].ap())
    ahst = tile1("ahst", (MROWS, SROWS))
    nc.scalar.dma_start(ahst[:, :], d["ahst"].ap())
    r0c = tile1("r0c", (1, 1))
    nc.scalar.dma_start(r0c[:, :], d["r0c"].ap())

    # engine-generated constants (no DMA)
    ones1 = tile1("ones1", (1, 128))
    nc.vector.memset(ones1[:, :], 1.0)
    id1 = tile1("id1", (1, 1))
    nc.vector.memset(id1[:, :], 1.0)
    row1x = tile1("row1x", (1, 38))
    nc.vector.memset(row1x[0:1, 37:38], 1.0)
    pio1i = tile1("pio1i", (128, 1), I32)
    nc.gpsimd.iota(pio1i[:, :], pattern=[[0, 1]], base=0, channel_multiplier=1)
    pio1f = tile1("pio1f")
    nc.vector.tensor_copy(pio1f[:, :], pio1i[:, :])
    pioei = tile1("pioei", (128, 1), I32)
    nc.gpsimd.iota(pioei[:, :], pattern=[[0, 1]], base=-int(BIG),
                   channel_multiplier=NPP)
    pioef = tile1("pioef")
    nc.vector.tensor_copy(pioef[:, :], pioei[:, :])
    xif = tile1("xif", (SWIN, WWIN))
    nc.scalar.dma_start(xif[:, :], d["xiota"].ap())

    # ---------------- stage S: score fusion + argmax ----------------
    P3 = P2[:, :].rearrange("p (n c) -> p n c", c=NC_COL)   # [128, 66, 37]

    sg = tile1("sg", (128, NPP))
    act(sg[:, :], P3[:, :, 4], AF.Sigmoid)
    s2 = tile1("s2", (128, NPP))
    ts(s2[:, :], sg[:, :], -0.5, 0.0, OP.add, OP.max)       # relu(sig-0.5)
    ts(s2[:, :], s2[:, :], 0.001, None, OP.add)

    mk = tile1("mk", (128, NPP))
    nc.vector.tensor_reduce(mk[:, :], P3[:, :, 5:NC_COL], AX.X, OP.add,
                            apply_absolute_value=True)

    dxa = tile1("dxa", (128, NPP))
    dya = tile1("dya", (128, NPP))
    act(dxa[:, :], P3[:, :, 0], AF.Abs, bias=-320.0, scale=640.0)
    act(dya[:, :], P3[:, :, 1], AF.Abs, bias=-320.0, scale=640.0)
    uxy = tile1("uxy", (128, NPP))
    tt(uxy[:, :], dxa[:, :], dya[:, :], OP.add)
    # 0.5 + 0.5*clamp(1 - uxy/640, 0, 1) == max(1 - uxy/1280, 0.5)
    cwf = tile1("cwf", (128, NPP))
    ts(cwf[:, :], uxy[:, :], -0.5 / 640.0, 1.0, OP.mult, OP.add)
    ts(cwf[:, :], cwf[:, :], 0.5, None, OP.max)

    score = tile1("score", (128, NPP))
    tt(score[:, :], s2[:, :], mk[:, :], OP.mult)
    tt(score[:, :], score[:, :], cwf[:, :], OP.mult)

    vmax8 = tile1("vmax8", (128, 8))
    vidx8 = tile1("vidx8", (128, 8), U32)
    nc.vector.max_with_indices(vmax8[:, :], vidx8[:, :], score[:, :])

    # winner anchor = min over {p : vmax[p] == global max} of (66p + idx[p]),
    # done in the transposed [1,128] domain with the -BIG encoding.
    afe = tile1("afe")
    nc.vector.tensor_copy(afe[:, :], vidx8[:, 0:1])
    ts(afe[:, :], afe[:, :], pioef[:, :], None, OP.add)     # 66p + idx - BIG
    pmA = ps.tile([1, 128], F32, tag="ps", name=f"pmA{rep}")
    nc.tensor.transpose(pmA[:, :], vmax8[:, 0:1], i128[:, :])
    pmB = ps.tile([1, 128], F32, tag="ps", name=f"pmB{rep}")
    nc.tensor.transpose(pmB[:, :], afe[:, :], i128[:, :])
    m11 = tile1("m11", (1, 1))
    nc.vector.tensor_reduce(m11[0:1, :], pmA[:, :], AX.X, OP.max)
    wm = tile1("wm", (1, 128))
    ts(wm[0:1, :], pmA[:, :], m11[0:1, :], None, OP.is_ge)
    cand = tile1("cand", (1, 128))
    tt(cand[0:1, :], pmB[:, :], wm[0:1, :], OP.mult)
    a_f = tile1("a_f", (1, 1))
    nc.vector.tensor_reduce(a_f[0:1, :], cand[0:1, :], AX.X, OP.min)
    ts(a_f[0:1, :], a_f[0:1, :], BIG, None, OP.add)
    a_i = tile1("a_i", (1, 1), I32)
    nc.vector.tensor_copy(a_i[0:1, :], a_f[0:1, :])

    if stage <= 1:
        metas = tile1("metas", (1, 8))
        nc.vector.memset(metas[:, :], 0.0)
        nc.vector.tensor_copy(metas[0:1, 0:1], a_f[0:1, :])
        nc.sync.dma_start(d["meta"].ap(), metas[:, :])
        ctx.close()
        return

    # ---------------- stage G: gather winner row; box -> windows ----------
    with nc.sync.register(f"aoff{rep}") as areg:
        nc.sync.reg_load(areg, a_i[0:1, 0:1])
        aoff = nc.sync.snap(areg, min_val=0, max_val=NANCH - 1)
        nc.sync.dma_start(row1x[:, 0:NC_COL],
                          d["pred"].ap()[bass.ds(aoff, 1), :])

    t38 = tile1("t38", (38, 1))
    psT = ps.tile([38, 1], F32, tag="ps", name=f"psT{rep}")
    nc.tensor.transpose(psT[:, :], row1x[:, :], id1[:, :])
    nc.scalar.copy(t38[:, :], psT[:, :])
    psC = ps.tile([32, 1], F32, tag="ps", name=f"psC{rep}")
    nc.tensor.transpose(psC[:, :], row1x[:, 5:NC_COL], id1[:, :])
    coefT = tile1("coefT", (32, 1))
    nc.scalar.copy(coefT[:, :], psC[:, :])
    coefT = coefT[:, :]

    psV = ps.tile([1, 8], F32, tag="ps", name=f"psV{rep}")
    nc.tensor.matmul(psV[:, :], t38[:, :], w38[:, :], start=True, stop=True)
    # vr = clamp(psV, 0, h8) * s8   (bias folded into w38 row 37)
    vr = tile1("vr", (1, 8))
    ts(vr[0:1, :], psV[:, :], 0.0, None, OP.max)
    tt(vr[0:1, :], vr[0:1, :], crow[0:1, 0:8], OP.min)
    tt(vr[0:1, :], vr[0:1, :], crow[0:1, 8:16], OP.mult)
    # vr = [fb0, fb1, fb2, fb3, m_pre, rw_pre, ww_pre, 0]
    # (ww from unrounded m_pre is safe: it can only yield ww or ww-1, both
    #  inside the 4-col slack.)

    ri4 = tile1("ri4", (1, 4), I32)                  # [m_i, rw_i, ww_i, c0_i]
    nc.vector.tensor_copy(ri4[0:1, 0:3], vr[0:1, 4:7])
    ts(ri4[0:1, 3:4], ri4[0:1, 0:1], 6, None, OP.mult)
    rf1 = tile1("rf1", (1, 1))
    nc.vector.tensor_copy(rf1[0:1, :], ri4[0:1, 1:2])
    c0_i = ri4[0:1, 3:4]

    # vrow2 = [fb0, fb1, fb2, fb3, c0, rw+r0, a, 0] -> meta + broadcast
    vrow2 = tile1("vrow2", (1, 8))
    nc.vector.memset(vrow2[:, :], 0.0)
    nc.vector.tensor_copy(vrow2[0:1, 0:4], vr[0:1, 0:4])
    nc.vector.tensor_copy(vrow2[0:1, 4:5], c0_i)
    tt(vrow2[0:1, 5:6], rf1[0:1, :], r0c[0:1, :], OP.add)
    nc.vector.tensor_copy(vrow2[0:1, 6:7], a_f[0:1, :])
    nc.sync.dma_start(d["meta"].ap(), vrow2[:, :])

    psF = ps.tile([128, 8], F32, tag="ps", name=f"psF{rep}")
    nc.tensor.matmul(psF[:, :], ones1[:, :], vrow2[:, :], start=True, stop=True)
    fbB = tile1("fbB", (128, 8))
    nc.scalar.copy(fbB[:, :], psF[:, :])

    riog = tile1("riog")                       # global row index per partition
    ts(riog[:, :], pio1f[:, :], fbB[:, 5:6], None, OP.add)
    rma = tile1("rma")
    rmb = tile1("rmb")
    ts(rma[:, :], riog[:, :], fbB[:, 1:2], None, OP.is_ge)
    ts(rmb[:, :], riog[:, :], fbB[:, 3:4], 255.0, OP.is_lt, OP.mult)
    rm255 = tile1("rm255")
    tt(rm255[:, :], rma[:, :], rmb[:, :], OP.mult)

    if stage <= 2:
        ctx.close()
        return

    # ---------------- stage M: windowed mask pipeline ----------------
    protosw = tile1("protosw", (32, WW160 * MROWS))
    awWf = tile1("awWf", (1, WW160 * SWIN))      # [1, (w m)] row layout
    vww = tile1("vww", (SWIN, WWIN))
    r82w = tile1("r82w", (SROWS, RWIN))
    xw = tile1("xw", (RWIN, WWIN * 3))

    # small window loads on the (idle) SP HWDGE queue
    sreg_ctx = contextlib.ExitStack()
    sm = sreg_ctx.enter_context(nc.sync.register(f"smo{rep}"))
    sr = sreg_ctx.enter_context(nc.sync.register(f"sro{rep}"))
    sw = sreg_ctx.enter_context(nc.sync.register(f"swo{rep}"))
    sc = sreg_ctx.enter_context(nc.sync.register(f"sco{rep}"))
    nc.sync.reg_load([sm, sr, sw, sc], ri4[0:1, 0:4])
    smo = nc.sync.snap(sm, min_val=0, max_val=(W0 - WWIN) // 6)
    sro = nc.sync.snap(sr, min_val=0, max_val=ROWS - RWIN)
    swo = nc.sync.snap(sw, min_val=0, max_val=160 - WW160)
    sco = nc.sync.snap(sc, min_val=0, max_val=W0 - WWIN)
    nc.sync.dma_start(
        awWf[:, :].rearrange("q (w m) -> q w m", m=SWIN),
        d["awtp"].ap()[bass.ds(swo, WW160), bass.ds(smo, SWIN)].unsqueeze(0))
    nc.sync.dma_start(
        vww[:, :], d["vwpad"].ap()[bass.ds(smo, SWIN), bass.ds(sco, WWIN)])
    nc.sync.dma_start(r82w[:, :], d["r82"].ap()[:, bass.ds(sro, RWIN)])
    sreg_ctx.close()

    greg_ctx = contextlib.ExitStack()
    wreg = greg_ctx.enter_context(nc.gpsimd.register(f"wo{rep}"))
    creg = greg_ctx.enter_context(nc.gpsimd.register(f"co_{rep}"))
    rreg = greg_ctx.enter_context(nc.gpsimd.register(f"ro{rep}"))
    nc.gpsimd.reg_load([rreg, wreg, creg], ri4[0:1, 1:4])
    wo = nc.gpsimd.snap(wreg, min_val=0, max_val=160 - WW160)
    co = nc.gpsimd.snap(creg, min_val=0, max_val=W0 - WWIN)
    ro = nc.gpsimd.snap(rreg, min_val=0, max_val=ROWS - RWIN)
    # proto is staged w-major ([32, w160, h24]) so this is 32 descriptors
    nc.gpsimd.dma_start(
        protosw[:, :].rearrange("c (w h) -> c w h", h=MROWS),
        d["protot"].ap().rearrange("c (w h) -> c w h", h=MROWS)
        [:, bass.ds(wo, WW160), :])
    nc.gpsimd.dma_start(
        xw[:, :].rearrange("p (w c) -> p w c", c=3),
        d["xs"].ap().rearrange("r (w c) -> r w c", c=3)
        [bass.ds(ro, RWIN), bass.ds(co, WWIN), :])

    # coef matvec over the window: m160 flat [1, (w8 h24)]
    psM = ps.tile([1, WW160 * MROWS], F32, tag="psM", name=f"psM{rep}", bufs=2)
    nc.tensor.matmul(psM[0:1, :], coefT, protosw[:, :], start=True, stop=True)
    m160wf = tile1("m160wf", (1, WW160 * MROWS))
    nc.scalar.copy(m160wf[:, :], psM[:, :])

    # step A: contract w via 8 accumulating rank-1 matmuls (no reshape DMA):
    #   sA[h24, i5] = sum_w m160wf[0, w*24:...][h] * awWf[0, w*5:...][i]
    psA = ps.tile([MROWS, SWIN], F32, tag="ps", name=f"psA{rep}")
    for w in range(WW160):
        nc.tensor.matmul(psA[:, :],
                         m160wf[0:1, MROWS * w:MROWS * (w + 1)],
                         awWf[0:1, SWIN * w:SWIN * (w + 1)],
                         start=(w == 0), stop=(w == WW160 - 1))
    sA = tile1("sA", (MROWS, SWIN))
    nc.scalar.copy(sA[:, :], psA[:, :])

    # step B: contract h:  m640T[i88, j82] = sum_h sA[h, i] * ahst[h, j]
    psB = ps.tile([SWIN, SROWS], F32, tag="ps", name=f"psB{rep}")
    nc.tensor.matmul(psB[:, :], sA[:, :], ahst[:, :], start=True, stop=True)
    s_winT = tile1("s_winT", (SWIN, SROWS))
    act(s_winT[:, :], psB[:, :], AF.Sigmoid)

    if stage <= 3:
        greg_ctx.close()
        ctx.close()
        return

    # ---------------- stage O: threshold + rect + multiply ----------------
    # column rect mask folded into the vww window (zeroed columns give
    # sigmoid-product 0 < MASK_THR)
    xcol = tile1("xcol", (SWIN, WWIN))
    ts(xcol[:, :], xif[:, :], fbB[0:SWIN, 4:5], None, OP.add)
    cma = tile1("cma", (SWIN, WWIN))
    ts(cma[:, :], xcol[:, :], fbB[0:SWIN, 0:1], None, OP.is_ge)
    cmb = tile1("cmb", (SWIN, WWIN))
    ts(cmb[:, :], xcol[:, :], fbB[0:SWIN, 2:3], None, OP.is_lt)
    colm = tile1("colm", (SWIN, WWIN))
    tt(colm[:, :], cma[:, :], cmb[:, :], OP.mult)
    vwwm = tile1("vwwm", (SWIN, WWIN))
    tt(vwwm[:, :], vww[:, :], colm[:, :], OP.mult)

    # step X: contract i: tX[j82, col] = sum_i s_winT[i, j] * vwwm[i, col]
    psX = ps.tile([SROWS, WWIN], F32, tag="psM", name=f"psX{rep}", bufs=2)
    nc.tensor.matmul(psX[:, :], s_winT[:, :], vwwm[:, :], start=True, stop=True)
    sX = tile1("sX", (SROWS, WWIN))
    nc.scalar.copy(sX[:, :], psX[:, :])
    # step W: contract j: m_orig[r8, col] = sum_j r82w[j, r] * tX[j, col]
    psW = ps.tile([RWIN, WWIN], F32, tag="psW", name=f"psW{rep}", bufs=2)
    nc.tensor.matmul(psW[:, :], r82w[:, :], sX[:, :], start=True, stop=True)
    bm3 = tile1("bm3", (RWIN, WWIN))
    ts(bm3[:, :], psW[:, :], MASK_THR, rm255[0:RWIN, :], OP.is_gt, OP.mult)

    res = tile1("res", (RWIN, 3 * WWIN))
    res3 = res[:, :].rearrange("p (w c) -> p w c", c=3)
    xw3 = xw[:, :].rearrange("p (w c) -> p w c", c=3)
    outv = d["out"].ap().rearrange("r (w c) -> r w c", c=3)
    try:
        bm3b = bm3[:, :].unsqueeze(2).broadcast_to((RWIN, WWIN, 3))
        tt(res3[:, :, :], xw3[:, :, :], bm3b, OP.mult)
    except Exception:
        for ch in range(3):
            tt(res3[:, :, ch], xw3[:, :, ch], bm3[:, :], OP.mult)
    nc.gpsimd.dma_start(
        outv[bass.ds(ro, RWIN), bass.ds(co, WWIN), :], res3[:, :, :])

    greg_ctx.close()
    ctx.close()


# ---------------------------------------------------------------------------
# host orchestration
# ---------------------------------------------------------------------------

_NC_CACHE = None


def _get_nc():
    global _NC_CACHE
    if _NC_CACHE is None:
        _NC_CACHE = _build_nc()
    return _NC_CACHE


def _make_in_maps(x_raw, pred2, proto2, shared, percore):
    import ml_dtypes
    predp = np.zeros((128, NPP * NC_COL), np.float32)
    predp.reshape(-1)[:NANCH * NC_COL] = pred2.reshape(-1)
    predp = predp.astype(ml_dtypes.bfloat16)
    in_maps = []
    for c in range(N_CORES):
        pc = percore[c]
        ha = pc["ha"]
        xs_cl = np.ascontiguousarray(
            x_raw[0, :, ROWS * c:ROWS * (c + 1), :].transpose(1, 2, 0)
        ).reshape(ROWS, W0 * 3)
        protot = np.ascontiguousarray(
            proto2[:, ha:ha + MROWS, :].transpose(0, 2, 1)
        ).reshape(32, 160 * MROWS)
        in_maps.append({
            "predp": predp,
            "pred": pred2,
            "xs": xs_cl,
            "protot": protot,
            "ahst": pc["ahst"],
            "awtp": shared["awtp"],
            "r82": pc["r82"],
            "vwpad": shared["vwpad"],
            "i128": shared["i128"],
            "xiota": shared["xiota"],
            "w38": pc["w38"],
            "crow": pc["crow"],
            "r0c": pc["r0c"],
        })
    return in_maps


def _numpy_fallback(x_raw, pred, proto):
    """Exact slow-path reference (only used if the rect exceeds the device
    windows, which cannot happen for in-distribution inputs)."""
    p = pred[0]
    boxes, cls, coef = p[:, :4], p[:, 4], p[:, 5:]
    s1 = np.maximum(1.0 / (1.0 + np.exp(-cls)) - 0.5, 0) + np.float32(0.001)
    mk = np.abs(coef).sum(-1)
    f = np.float32(640.0 if boxes.max() <= 1.2 else 1.0)
    dxdy = np.abs(boxes[:, :2] * f - 320.0) / 320.0
    cw = np.maximum(1.0 - 0.5 * (dxdy[:, 0] + dxdy[:, 1]), 0.0)
    a = int(np.argmax(s1 * mk * (0.5 + 0.5 * cw)))
    fcoef = coef[a]
    cx, cy, w, h = boxes[a]
    xyxy = np.clip(np.array([cx - w / 2, cy - h / 2, cx + w / 2, cy + h / 2],
                            np.float32), 0.0, IMGSZ - 1)
    fb = xyxy * np.array([W0 / IMGSZ, H0 / IMGSZ, W0 / IMGSZ, H0 / IMGSZ],
                         np.float32)
    Ah = _weight_mat(160, IMGSZ)
    Aw = _weight_mat(160, IMGSZ)
    Vh = _weight_mat(IMGSZ, H0)
    Vw = _weight_mat(IMGSZ, W0)
    m160 = (fcoef @ proto[0].reshape(32, -1)).reshape(160, 160)
    m640 = Ah.T @ m160 @ Aw
    s640 = 1.0 / (1.0 + np.exp(-m640))
    m_orig = (Vh.T @ s640 @ Vw).astype(np.float32)
    ys = np.arange(H0, dtype=np.float32)[:, None]
    xs = np.arange(W0, dtype=np.float32)[None, :]
    rect = (xs >= fb[0]) & (xs < fb[2]) & (ys >= fb[1]) & (ys < fb[3])
    bm = ((m_orig > MASK_THR) & rect).astype(np.float32)
    return (np.clip(x_raw * 255.0, 0.0, 255.0) * bm[None, None]).astype(np.float32)


def _covered(metas):
    """Check every rect pixel lies inside each core's written window.
    meta = [fb0, fb1, fb2, fb3, c0, rw + r0, a, 0]"""
    fb0, fb1, fb2, fb3 = metas[0][0], metas[0][1], metas[0][2], metas[0][3]
    if fb2 <= fb0 or fb3 <= fb1:
        return True
    c0 = metas[0][4]
    cols = np.arange(W0, dtype=np.float32)
    csel = (cols >= fb0) & (cols < fb2)
    if csel.any():
        lo, hi = np.where(csel)[0][[0, -1]]
        if not (c0 <= lo and hi < c0 + WWIN):
            return False
    rows = np.arange(H0, dtype=np.float32)
    rsel = (rows >= fb1) & (rows < fb3)
    for c in range(N_CORES):
        sel = rsel[ROWS * c:ROWS * (c + 1)]
        if sel.any():
            rw = metas[c][5] - ROWS * c
            lo, hi = np.where(sel)[0][[0, -1]]
            if not (rw <= lo and hi < rw + RWIN):
                return False
    return True


def _host_score_argmax(p):
    boxes, cls, coef = p[:, :4], p[:, 4], p[:, 5:]
    s1 = np.maximum(1.0 / (1.0 + np.exp(-cls)) - 0.5, 0) + np.float32(0.001)
    mk = np.abs(coef).sum(-1)
    dxdy = np.abs(boxes[:, :2] * 640.0 - 320.0) / 320.0
    cw = np.maximum(1.0 - 0.5 * (dxdy[:, 0] + dxdy[:, 1]), 0.0)
    return int(np.argmax(s1 * mk * (0.5 + 0.5 * cw)))


def kernel(x_raw, pred, proto):
    import ml_dtypes
    x_raw = np.ascontiguousarray(np.asarray(x_raw, dtype=np.float32))
    pred = np.ascontiguousarray(np.asarray(pred, dtype=np.float32))
    proto = np.ascontiguousarray(np.asarray(proto, dtype=np.float32))

    if float(pred[0, :, :4].max()) > 1.2:
        # device hardcodes the is_norm=True 640x scaling
        return _numpy_fallback(x_raw, pred, proto)
    # the device scores bf16-quantized pred; bail out if that could change
    # the winner
    predq = pred[0].astype(ml_dtypes.bfloat16).astype(np.float32)
    if _host_score_argmax(pred[0]) != _host_score_argmax(predq):
        return _numpy_fallback(x_raw, pred, proto)

    nc = _get_nc()
    shared, percore = _host_consts()
    pred2 = np.ascontiguousarray(pred[0])
    proto2 = proto[0]
    in_maps = _make_in_maps(x_raw, pred2, proto2, shared, percore)

    res = bass_utils.run_bass_kernel_spmd(nc, in_maps,
                                          core_ids=list(range(N_CORES)))

    metas = [res.results[c]["meta"][0] for c in range(N_CORES)]
    if not _covered(metas):
        return _numpy_fallback(x_raw, pred, proto)

    out = np.concatenate(
        [res.results[c]["out"].reshape(ROWS, W0, 3) for c in range(N_CORES)],
        axis=0)                                   # [2160, 3840, 3]
    return np.ascontiguousarray(out.transpose(2, 0, 1))[None]


if __name__ == "__main__":
    import jax
    with jax.default_device(jax.devices("cpu")[0]):
        import reference as R
        inputs = R.setup_inputs()
        inputs = {k: np.asarray(v) for k, v in inputs.items()}
    out = kernel(**inputs)
    ref = np.load("/tmp/ref_out.npy")
    print("absmax:", np.abs(out - ref).max())
